# revision 7
# baseline (speedup 1.0000x reference)
"""Trainium2 Bass kernel for ConvFCNet (3x conv+pool -> int8-fakequant FC + LIF SNN head).

Data-parallel over 8 NeuronCores: batch 512 -> 64 samples/core, weights replicated.

Per-core pipeline (all activations bf16, PSUM accumulation fp32, LIF state fp32):
  conv1 3->32  48x48, pad1 + relu + maxpool2 -> [32, 24, 24]
      im2col (27 = 3c x 9 taps) built by DMA into 4 partition groups of 32,
      4 concurrent diagonal tile_position matmuls (K=27, M=32).
  conv2 32->64 24x24 -> [64, 12, 12]
      im2col over dx only (96 = 32c x 3dx); dy folded into matmul free-dim offsets;
      3 accumulating K=96 matmuls, 2 sample-halves run in parallel via col tiling.
  conv3 64->128 12x12 -> [128, 6, 6]
      im2col over dx: A=[128 = 64c x dx01], B=[64 = c, dx2]; 6 accumulating matmuls.
  FC1 4608->512 per-unit-group (4x128) stationary weights, feat chunks (hw-major) as rhs.
  LIF dynamics (tau=2, vth=1, hard reset) + FC2 512->128 + FC3 128->5, 3 timesteps,
      all in [unit, sample] orientation -> no transposes.
"""

import numpy as np
import ml_dtypes

import concourse.bass as bass
import concourse.bacc as bacc
import concourse.tile as tile
import concourse.mybir as mybir

AF = mybir.ActivationFunctionType
ALU = mybir.AluOpType
BF16 = mybir.dt.bfloat16
F32 = mybir.dt.float32

NCORES = 8
B = 64  # samples per core


def _v(ap, p0, npart, dims, off=0):
    """View into an SBUF/PSUM tile AP: partition slice [p0, p0+npart) + custom free dims."""
    pitch = ap.ap[0][0]
    return bass.AP(
        tensor=ap.tensor,
        offset=ap.offset + p0 * pitch + off,
        ap=[[pitch, npart]] + [list(d) for d in dims],
    )


def _dv(ap, off, dims):
    """View into a DRAM tensor AP with custom dims."""
    return bass.AP(tensor=ap.tensor, offset=ap.offset + off, ap=[list(d) for d in dims])


def _emit(tc, io):
    nc = tc.nc
    from contextlib import ExitStack

    with ExitStack() as ctx:
        # ---------------- persistent buffers + weights ----------------
        wp = ctx.enter_context(tc.tile_pool(name="wts", bufs=1))
        w1sb = wp.tile([128, 32], BF16)
        nc.sync.dma_start(w1sb[:, :], io["w1l"][:, :])
        w2sb = wp.tile([96, 192], BF16)
        nc.sync.dma_start(w2sb[:, :], io["w2l"][:, :])
        w3asb = wp.tile([128, 384], BF16)
        nc.sync.dma_start(w3asb[:, :], io["w3a"][:, :])
        w3bsb = wp.tile([64, 384], BF16)
        nc.sync.dma_start(w3bsb[:, :], io["w3b"][:, :])
        wf2sb = wp.tile([128, 512], BF16)
        nc.sync.dma_start(wf2sb[:, :], io["wf2"][:, :])
        wf3sb = wp.tile([128, 5], BF16)
        nc.sync.dma_start(wf3sb[:, :], io["wf3"][:, :])
        b1sb = wp.tile([128, 1], F32)
        nc.sync.dma_start(b1sb[:, :], io["b1"][:, :])
        b2sb = wp.tile([128, 1], F32)
        nc.sync.dma_start(b2sb[:, :], io["b2"][:, :])
        b3sb = wp.tile([128, 1], F32)
        nc.sync.dma_start(b3sb[:, :], io["b3"][:, :])

        mp = ctx.enter_context(tc.tile_pool(name="main", bufs=1))
        # conv1 pooled output, padded 26x26, partition 32q+c holds samples 16q..16q+15
        xpad2 = mp.tile([128, 16 * 676 + 4], BF16)
        nc.gpsimd.memset(xpad2[:, :], 0.0)
        # conv2 pooled output, padded 14x14, partition 64h+c holds samples 32h..32h+31
        xpad3 = mp.tile([128, 32 * 198 + 4], BF16)
        nc.gpsimd.memset(xpad3[:, :], 0.0)
        # conv3 pooled output (features): [128c, b*36 + hw]
        feat = mp.tile([128, B * 36], BF16)

        # ---------------- conv1 ----------------
        with (
            tc.tile_pool(name="c1imc", bufs=2) as c1i,
            tc.tile_pool(name="c1ps", bufs=4, space="PSUM") as c1p,
            tc.tile_pool(name="c1t", bufs=3) as c1t,
        ):
            for half in range(2):
                imc = c1i.tile([128, 8 * 2500], BF16, tag="imc")
                for q in range(4):
                    for dy in range(3):
                        src = _dv(
                            io["xpad"],
                            (q * 16 + half * 8) * 2500 + dy * 50,
                            [[1, 3], [160128, 3], [1, 20000]],
                        )
                        dst = _v(imc, 32 * q + dy * 9, 9, [[1, 20000]])
                        nc.sync.dma_start(dst, src)
                for s in range(8):
                    for yt in range(6):
                        ps = c1p.tile([128, 384], F32, tag="ps1")
                        for q in range(4):
                            rhs = _v(imc, 32 * q, 27, [[50, 8], [1, 48]], s * 2500 + yt * 400)
                            nc.tensor.matmul(
                                ps[32 * q : 32 * q + 32, :],
                                w1sb[32 * q : 32 * q + 27, 0:32],
                                rhs,
                                start=True,
                                stop=True,
                                tile_position=(32 * q, 32 * q),
                            )
                        # maxpool 2x2 on (8y, 48x) -> (4y, 24x), then relu+bias -> xpad2
                        t2 = c1t.tile([128, 96], BF16, tag="t2")
                        nc.vector.tensor_reduce(
                            _v(t2, 0, 128, [[24, 4], [1, 24]]),
                            _v(ps, 0, 128, [[96, 4], [2, 24], [48, 2], [1, 2]]),
                            mybir.AxisListType.XY,
                            ALU.max,
                        )
                        hs = half * 8 + s
                        dst = _v(xpad2, 0, 128, [[26, 4], [1, 24]], hs * 676 + 27 + yt * 4 * 26)
                        nc.scalar.activation(dst, t2[:, :], AF.Relu, bias=b1sb[:, 0:1])

        # ---------------- conv2 ----------------
        with (
            tc.tile_pool(name="c2buf", bufs=1) as c2b,
            tc.tile_pool(name="c2ps", bufs=4, space="PSUM") as c2p,
            tc.tile_pool(name="c2t", bufs=3) as c2t,
        ):
            buf96 = c2b.tile([96, B * 676 + 4], BF16)
            for j in range(4):
                src = _v(xpad2, 32 * j, 32, [[1, 3], [1, 16 * 676]])
                dst = _v(buf96, 0, 96, [[1, 16 * 676]], j * 16 * 676)
                nc.sync.dma_start(dst, src)
            for b in range(32):
                for yh in range(2):
                    ps = c2p.tile([128, 288], F32, tag="ps2")
                    for h in range(2):
                        for dy in range(3):
                            rhs = _v(
                                buf96, 0, 96, [[26, 12], [1, 24]],
                                (h * 32 + b) * 676 + yh * 312 + dy * 26,
                            )
                            nc.tensor.matmul(
                                ps[64 * h : 64 * h + 64, :],
                                w2sb[0:96, dy * 64 : dy * 64 + 64],
                                rhs,
                                start=(dy == 0),
                                stop=(dy == 2),
                                tile_position=(0, 64 * h),
                            )
                    # pool (12y, 24x) -> (6y, 12x)
                    t2 = c2t.tile([128, 72], BF16, tag="t2")
                    nc.vector.tensor_reduce(
                        _v(t2, 0, 128, [[12, 6], [1, 12]]),
                        _v(ps, 0, 128, [[48, 6], [2, 12], [24, 2], [1, 2]]),
                        mybir.AxisListType.XY,
                        ALU.max,
                    )
                    dst = _v(xpad3, 0, 128, [[14, 6], [1, 12]], b * 198 + 15 + yh * 84)
                    nc.scalar.activation(dst, t2[:, :], AF.Relu, bias=b2sb[:, 0:1])

        # ---------------- conv3 ----------------
        with (
            tc.tile_pool(name="c3buf", bufs=2) as c3b,
            tc.tile_pool(name="c3ps", bufs=4, space="PSUM") as c3p,
            tc.tile_pool(name="c3t", bufs=3) as c3t,
        ):
            for h in range(2):
                bufA = c3b.tile([128, 32 * 198 + 4], BF16, tag="A")
                bufB = c3b.tile([64, 32 * 198 + 4], BF16, tag="B")
                nc.sync.dma_start(
                    _v(bufA, 0, 128, [[1, 32 * 198]]),
                    _v(xpad3, 64 * h, 64, [[1, 2], [1, 32 * 198]]),
                )
                nc.sync.dma_start(
                    _v(bufB, 0, 64, [[1, 32 * 198]]),
                    _v(xpad3, 64 * h, 64, [[1, 32 * 198]], 2),
                )
                for bp in range(16):
                    ps = c3p.tile([128, 288], F32, tag="ps3")
                    for dy in range(3):
                        dims = [[198, 2], [14, 12], [1, 12]]
                        off = bp * 2 * 198 + dy * 14
                        nc.tensor.matmul(
                            ps[:, :], w3asb[0:128, dy * 128 : dy * 128 + 128],
                            _v(bufA, 0, 128, dims, off),
                            start=(dy == 0), stop=False,
                        )
                        nc.tensor.matmul(
                            ps[:, :], w3bsb[0:64, dy * 128 : dy * 128 + 128],
                            _v(bufB, 0, 64, dims, off),
                            start=False, stop=(dy == 2),
                        )
                    # pool (2b, 12y, 12x) -> (2b, 6y, 6x): one XY-reduce per sample
                    t2 = c3t.tile([128, 72], BF16, tag="t2")
                    for i in range(2):
                        nc.vector.tensor_reduce(
                            _v(t2, 0, 128, [[6, 6], [1, 6]], i * 36),
                            _v(ps, 0, 128, [[24, 6], [2, 6], [12, 2], [1, 2]], i * 144),
                            mybir.AxisListType.XY,
                            ALU.max,
                        )
                    dst = _v(feat, 0, 128, [[36, 2], [1, 36]], (h * 32 + 2 * bp) * 36)
                    nc.scalar.activation(dst, t2[:, :], AF.Relu, bias=b3sb[:, 0:1])

        # ---------------- FC1 + LIF + FC2/FC3 ----------------
        with (
            tc.tile_pool(name="fcw", bufs=1) as fcw,
            tc.tile_pool(name="cur1p", bufs=1, space="PSUM") as cur1p,
            tc.tile_pool(name="cur2p", bufs=2, space="PSUM") as cur2p,
            tc.tile_pool(name="lif", bufs=1) as lifp,
            tc.tile_pool(name="liftmp", bufs=2) as dtp,
        ):
            wf1sb = fcw.tile([128, 18432], BF16)
            nc.sync.dma_start(wf1sb[:, :], io["wf1"][:, :])

            cur1 = []
            for g in range(4):
                pt = cur1p.tile([128, 64], F32, tag=f"cur1_{g}", name=f"cur1_{g}")
                cur1.append(pt)
                for k in range(36):
                    col = (g * 36 + k) * 128
                    nc.tensor.matmul(
                        pt[:, :],
                        wf1sb[:, col : col + 128],
                        _v(feat, 0, 128, [[36, 64]], k),
                        start=(k == 0),
                        stop=(k == 35),
                    )

            zeros64 = lifp.tile([128, 64], F32)
            nc.gpsimd.memset(zeros64[:, :], 0.0)
            v1 = [lifp.tile([128, 64], F32, tag=f"v1_{g}", name=f"v1_{g}") for g in range(4)]
            s1 = [lifp.tile([128, 64], BF16, tag=f"s1_{g}", name=f"s1_{g}") for g in range(4)]
            for g in range(4):
                nc.gpsimd.memset(v1[g][:, :], 0.0)
            v2 = lifp.tile([128, 64], F32)
            nc.gpsimd.memset(v2[:, :], 0.0)
            s2 = lifp.tile([128, 64], BF16)
            v3 = lifp.tile([5, 64], F32)
            nc.gpsimd.memset(v3[:, :], 0.0)
            acc = lifp.tile([5, 64], F32)
            nc.gpsimd.memset(acc[:, :], 0.0)

            def lif_step(v, cur, s_out):
                # v <- v + (cur - v)*0.5 ; s = (v >= 1) ; v <- 0 where s
                d = dtp.tile(list(v.shape), F32, tag="d", name="d")
                nc.vector.tensor_tensor(d[:, :], cur[:, :], v[:, :], ALU.subtract)
                nc.vector.scalar_tensor_tensor(v[:, :], d[:, :], 0.5, v[:, :], ALU.mult, ALU.add)
                nc.vector.tensor_scalar(s_out[:, :], v[:, :], 1.0, None, ALU.is_ge)
                mask = s_out[:, :].bitcast(mybir.dt.uint16 if s_out.dtype == BF16 else mybir.dt.uint32)
                nc.vector.copy_predicated(v[:, :], mask, zeros64[0 : v.shape[0], :])

            for t in range(3):
                for g in range(4):
                    lif_step(v1[g], cur1[g], s1[g])
                cur2 = cur2p.tile([128, 64], F32, tag="cur2")
                for g in range(4):
                    nc.tensor.matmul(
                        cur2[:, :], wf2sb[:, g * 128 : g * 128 + 128], s1[g][:, :],
                        start=(g == 0), stop=(g == 3),
                    )
                lif_step(v2, cur2, s2)
                cur3 = cur2p.tile([5, 64], F32, tag="cur3")
                nc.tensor.matmul(cur3[0:5, :], wf3sb[0:128, 0:5], s2[:, :], start=True, stop=True)
                s3 = dtp.tile([5, 64], F32, tag="s3")
                d3 = dtp.tile([5, 64], F32, tag="d3")
                nc.vector.tensor_tensor(d3[:, :], cur3[0:5, :], v3[:, :], ALU.subtract)
                nc.vector.scalar_tensor_tensor(v3[:, :], d3[:, :], 0.5, v3[:, :], ALU.mult, ALU.add)
                nc.vector.tensor_scalar(s3[:, :], v3[:, :], 1.0, None, ALU.is_ge)
                nc.vector.copy_predicated(v3[:, :], s3[:, :].bitcast(mybir.dt.uint32), zeros64[0:5, :])
                nc.vector.tensor_tensor(acc[:, :], acc[:, :], s3[:, :], ALU.add)

            # acc/3 for acc in {0,1,2,3}: mult by fp32(1/3) matches true division except acc=3
            # (3*0.33333334 = 1.0000001) -> clamp with min(., 1.0) for exactness.
            nc.vector.tensor_scalar(acc[:, :], acc[:, :], float(np.float32(1.0) / np.float32(3.0)), 1.0, ALU.mult, ALU.min)
            nc.sync.dma_start(_dv(io["out"], 0, [[64, 5], [1, 64]]), acc[:, :])


def _build():
    nc = bacc.Bacc("TRN2", target_bir_lowering=False, debug=False, enable_asserts=True)
    io = {}

    def inp(name, shape, dt):
        io[name] = nc.dram_tensor(name, shape, dt, kind="ExternalInput").ap()

    inp("xpad", [3, 160128], BF16)
    inp("w1l", [128, 32], BF16)
    inp("w2l", [96, 192], BF16)
    inp("w3a", [128, 384], BF16)
    inp("w3b", [64, 384], BF16)
    inp("wf1", [128, 18432], BF16)
    inp("wf2", [128, 512], BF16)
    inp("wf3", [128, 5], BF16)
    inp("b1", [128, 1], F32)
    inp("b2", [128, 1], F32)
    inp("b3", [128, 1], F32)
    io["out"] = nc.dram_tensor("out", [5, 64], F32, kind="ExternalOutput").ap()

    with tile.TileContext(nc) as tc:
        _emit(tc, io)
    nc.compile()
    return nc


def _fake_quant(w):
    w = np.asarray(w, np.float32)
    scale = np.float32(np.max(np.abs(w)) / np.float32(127.0))
    wq = np.clip(np.round(w / scale), -127.0, 127.0).astype(np.float32) * scale
    return wq.astype(np.float32)


def _bf16(a):
    return np.asarray(a, np.float32).astype(ml_dtypes.bfloat16)


def _prep_weights(conv1_w, conv1_b, conv2_w, conv2_b, conv3_w, conv3_b, W1, W2, W3):
    c1 = np.asarray(conv1_w, np.float32)  # [32, 3, 3, 3]
    c2 = np.asarray(conv2_w, np.float32)  # [64, 32, 3, 3]
    c3 = np.asarray(conv3_w, np.float32)  # [128, 64, 3, 3]

    w1l = np.zeros((128, 32), np.float32)
    wk = c1.transpose(2, 3, 1, 0).reshape(27, 32)  # [(dy,dx,c), m]
    for q in range(4):
        w1l[32 * q : 32 * q + 27] = wk

    w2l = c2.transpose(1, 3, 2, 0).reshape(96, 192)  # [(c,dx), (dy,m)]
    w3x = c3.transpose(1, 3, 2, 0)  # [c, dx, dy, m]
    w3a = w3x[:, 0:2].reshape(128, 384)
    w3b = w3x[:, 2].reshape(64, 384)

    W1q = _fake_quant(W1)  # [512, 4608]
    W2q = _fake_quant(W2)  # [128, 512]
    W3q = _fake_quant(W3)  # [5, 128]

    wf1 = W1q.reshape(4, 128, 128, 36).transpose(2, 0, 3, 1).reshape(128, 4 * 36 * 128)
    wf2 = W2q.T.reshape(4, 128, 128).transpose(1, 0, 2).reshape(128, 512)
    wf3 = W3q.T.copy()  # [128, 5]

    return {
        "w1l": _bf16(w1l),
        "w2l": _bf16(w2l),
        "w3a": _bf16(w3a),
        "w3b": _bf16(w3b),
        "wf1": _bf16(wf1),
        "wf2": _bf16(wf2),
        "wf3": _bf16(wf3),
        "b1": np.tile(np.asarray(conv1_b, np.float32), 4).reshape(128, 1).copy(),
        "b2": np.tile(np.asarray(conv2_b, np.float32), 2).reshape(128, 1).copy(),
        "b3": np.asarray(conv3_b, np.float32).reshape(128, 1).copy(),
    }


_NC = None
LAST_RESULTS = None


def kernel(x, conv1_w, conv1_b, conv2_w, conv2_b, conv3_w, conv3_b, W1, W2, W3, _trace=False):
    global _NC, LAST_RESULTS
    if _NC is None:
        _NC = _build()

    wmap = _prep_weights(conv1_w, conv1_b, conv2_w, conv2_b, conv3_w, conv3_b, W1, W2, W3)

    x = np.asarray(x, np.float32)
    xp = np.zeros((512, 3, 50, 50), np.float32)
    xp[:, :, 1:49, 1:49] = x
    in_maps = []
    for i in range(NCORES):
        shard = xp[B * i : B * (i + 1)].transpose(1, 0, 2, 3).reshape(3, B * 2500)
        sp = np.zeros((3, 160128), np.float32)
        sp[:, : B * 2500] = shard
        in_maps.append({"xpad": _bf16(sp), **wmap})

    from concourse.bass_utils import run_bass_kernel_spmd

    res = run_bass_kernel_spmd(_NC, in_maps, core_ids=list(range(NCORES)), trace=_trace)
    LAST_RESULTS = res
    out = np.concatenate([np.asarray(res.results[i]["out"]).T for i in range(NCORES)], axis=0)
    return np.ascontiguousarray(out.astype(np.float32))


# revision 16
# speedup vs baseline: 390.4042x; 390.4042x over previous
"""Trainium2 Bass kernel for ConvFCNet (3x conv+pool -> int8-fakequant FC + LIF SNN head).

Data-parallel over 8 NeuronCores: batch 512 -> 64 samples/core, weights replicated.

Per-core pipeline (all activations bf16, PSUM accumulation fp32, LIF state fp32):
  conv1 3->32  48x48, pad1 + relu + maxpool2 -> [32, 24, 24]
      im2col (27 = 3c x 9 taps) built by DMA into 4 partition groups of 32,
      4 concurrent diagonal tile_position matmuls (K=27, M=32).
  conv2 32->64 24x24 -> [64, 12, 12]
      im2col over dx only (96 = 32c x 3dx); dy folded into matmul free-dim offsets;
      3 accumulating K=96 matmuls, 2 sample-halves run in parallel via col tiling.
  conv3 64->128 12x12 -> [128, 6, 6]
      im2col over dx: A=[128 = 64c x dx01], B=[64 = c, dx2]; 6 accumulating matmuls.
  FC1 4608->512 per-unit-group (4x128) stationary weights, feat chunks (hw-major) as rhs.
  LIF dynamics (tau=2, vth=1, hard reset) + FC2 512->128 + FC3 128->5, 3 timesteps,
      all in [unit, sample] orientation -> no transposes.
"""

import numpy as np
import ml_dtypes

import concourse.bass as bass
import concourse.bacc as bacc
import concourse.tile as tile
import concourse.mybir as mybir

AF = mybir.ActivationFunctionType
ALU = mybir.AluOpType
BF16 = mybir.dt.bfloat16
F32 = mybir.dt.float32

NCORES = 8
B = 64  # samples per core


def _v(ap, p0, npart, dims, off=0):
    """View into an SBUF/PSUM tile AP: partition slice [p0, p0+npart) + custom free dims."""
    pitch = ap.ap[0][0]
    return bass.AP(
        tensor=ap.tensor,
        offset=ap.offset + p0 * pitch + off,
        ap=[[pitch, npart]] + [list(d) for d in dims],
    )


def _dv(ap, off, dims):
    """View into a DRAM tensor AP with custom dims."""
    return bass.AP(tensor=ap.tensor, offset=ap.offset + off, ap=[list(d) for d in dims])


def _emit(tc, io):
    nc = tc.nc
    from contextlib import ExitStack

    with ExitStack() as ctx:
        # ---------------- persistent buffers + weights ----------------
        wp = ctx.enter_context(tc.tile_pool(name="wts", bufs=1))
        w1sb = wp.tile([128, 32], BF16)
        nc.sync.dma_start(w1sb[:, :], io["w1l"][:, :])
        w2sb = wp.tile([96, 192], BF16)
        nc.sync.dma_start(w2sb[:, :], io["w2l"][:, :])
        w3asb = wp.tile([128, 384], BF16)
        nc.sync.dma_start(w3asb[:, :], io["w3a"][:, :])
        w3bsb = wp.tile([64, 384], BF16)
        nc.sync.dma_start(w3bsb[:, :], io["w3b"][:, :])
        wf2sb = wp.tile([128, 512], BF16)
        nc.sync.dma_start(wf2sb[:, :], io["wf2"][:, :])
        wf3sb = wp.tile([128, 5], BF16)
        nc.sync.dma_start(wf3sb[:, :], io["wf3"][:, :])
        identsb = wp.tile([64, 64], BF16)
        nc.sync.dma_start(identsb[:, :], io["ident"][:, :])
        b1sb = wp.tile([128, 1], F32)
        nc.sync.dma_start(b1sb[:, :], io["b1"][:, :])
        b2sb = wp.tile([128, 1], F32)
        nc.sync.dma_start(b2sb[:, :], io["b2"][:, :])
        b3sb = wp.tile([128, 1], F32)
        nc.sync.dma_start(b3sb[:, :], io["b3"][:, :])

        mp = ctx.enter_context(tc.tile_pool(name="main", bufs=1))
        # conv1 pooled output, padded 26x26, partition 32q+c holds samples 16q..16q+15
        xpad2 = mp.tile([128, 16 * 676 + 4], BF16)
        for dims, off in [
            ([[676, 16], [1, 26]], 0),        # top row
            ([[676, 16], [1, 26]], 650),      # bottom row
            ([[676, 16], [26, 26]], 0),       # left col
            ([[676, 16], [26, 26]], 25),      # right col
            ([[1, 4]], 16 * 676),             # tail pad (im2col dx over-read)
        ]:
            nc.gpsimd.memset(_v(xpad2, 0, 128, dims, off), 0.0)
        # conv2 pooled output, padded 14x14, partition 64h+c holds samples 32h..32h+31
        xpad3 = mp.tile([128, 32 * 198 + 4], BF16)
        for dims, off in [
            ([[198, 32], [1, 14]], 0),        # top row
            ([[198, 32], [1, 14]], 182),      # bottom row
            ([[198, 32], [14, 14]], 0),       # left col
            ([[198, 32], [14, 14]], 13),      # right col
            ([[1, 4]], 32 * 198),             # tail pad (im2col dx over-read)
            ([[198, 32], [1, 2]], 196),       # per-sample slack (pitch 198 vs 196)
        ]:
            nc.gpsimd.memset(_v(xpad3, 0, 128, dims, off), 0.0)
        # conv3 pooled output (features): [128c, b*36 + hw]
        feat = mp.tile([128, B * 36], BF16)

        # ---------------- conv1 ----------------
        with (
            tc.tile_pool(name="c1imc", bufs=3) as c1i,
            tc.tile_pool(name="c1ps", bufs=4, space="PSUM") as c1p,
            tc.tile_pool(name="c1t", bufs=3) as c1t,
        ):
            for half in range(4):
                imc = c1i.tile([128, 4 * 2500], BF16, tag="imc")
                for q in range(4):
                    for dy in range(3):
                        src = _dv(
                            io["xpad"],
                            (q * 16 + half * 4) * 2500 + dy * 50,
                            [[1, 3], [160128, 3], [1, 10000]],
                        )
                        dst = _v(imc, 32 * q + dy * 9, 9, [[1, 10000]])
                        nc.sync.dma_start(dst, src)
                for s in range(4):
                    stg = c1t.tile([128, 576], BF16, tag="stg")
                    for yt in range(6):
                        ps = c1p.tile([128, 384], F32, tag="ps1")
                        for q in range(4):
                            rhs = _v(imc, 32 * q, 27, [[50, 8], [1, 48]], s * 2500 + yt * 400)
                            nc.tensor.matmul(
                                ps[32 * q : 32 * q + 32, :],
                                w1sb[32 * q : 32 * q + 27, 0:32],
                                rhs,
                                start=True,
                                stop=True,
                                tile_position=(32 * q, 32 * q),
                            )
                        # maxpool 2x2 on (8y, 48x) -> (4y, 24x) into the staging tile
                        nc.vector.tensor_reduce(
                            _v(stg, 0, 128, [[24, 4], [1, 24]], yt * 96),
                            _v(ps, 0, 128, [[96, 4], [2, 24], [48, 2], [1, 2]]),
                            mybir.AxisListType.XY,
                            ALU.max,
                        )
                    hs = half * 4 + s
                    dst = _v(xpad2, 0, 128, [[26, 24], [1, 24]], hs * 676 + 27)
                    nc.scalar.activation(dst, _v(stg, 0, 128, [[24, 24], [1, 24]]), AF.Relu, bias=b1sb[:, 0:1])

        # FC1 weights: start the 4.7MB load early so it overlaps conv2/conv3 compute
        fcw = ctx.enter_context(tc.tile_pool(name="fcw", bufs=1))
        wf1sb = fcw.tile([128, 18432], BF16)
        nc.sync.dma_start(wf1sb[:, :], io["wf1"][:, :])

        # ---------------- conv2 ----------------
        with (
            tc.tile_pool(name="c2buf", bufs=1) as c2b,
            tc.tile_pool(name="c2ps", bufs=4, space="PSUM") as c2p,
            tc.tile_pool(name="c2t", bufs=3) as c2t,
        ):
            buf96 = c2b.tile([96, B * 676 + 4], BF16)
            for qt in range(4):
                for j in range(4):
                    src = _v(xpad2, 32 * j, 32, [[1, 3], [1, 4 * 676]], qt * 4 * 676)
                    dst = _v(buf96, 0, 96, [[1, 4 * 676]], (j * 16 + qt * 4) * 676)
                    nc.sync.dma_start(dst, src)
            for blk in range(8):
                stg = c2t.tile([128, 576], BF16, tag="stg")
                for bi in range(4):
                    b = blk * 4 + bi
                    for yh in range(2):
                        ps = c2p.tile([128, 288], F32, tag="ps2")
                        for h in range(2):
                            for dy in range(3):
                                rhs = _v(
                                    buf96, 0, 96, [[26, 12], [1, 24]],
                                    (h * 32 + b) * 676 + yh * 312 + dy * 26,
                                )
                                nc.tensor.matmul(
                                    ps[64 * h : 64 * h + 64, :],
                                    w2sb[0:96, dy * 64 : dy * 64 + 64],
                                    rhs,
                                    start=(dy == 0),
                                    stop=(dy == 2),
                                    tile_position=(0, 64 * h),
                                )
                        # pool (12y, 24x) -> (6y, 12x) into staging
                        nc.vector.tensor_reduce(
                            _v(stg, 0, 128, [[12, 6], [1, 12]], bi * 144 + yh * 72),
                            _v(ps, 0, 128, [[48, 6], [2, 12], [24, 2], [1, 2]]),
                            mybir.AxisListType.XY,
                            ALU.max,
                        )
                dst = _v(xpad3, 0, 128, [[198, 4], [14, 12], [1, 12]], blk * 4 * 198 + 15)
                nc.scalar.activation(dst, _v(stg, 0, 128, [[144, 4], [12, 12], [1, 12]]), AF.Relu, bias=b2sb[:, 0:1])

        # ---------------- conv3 ----------------
        with (
            tc.tile_pool(name="c3buf", bufs=2) as c3b,
            tc.tile_pool(name="c3ps", bufs=4, space="PSUM") as c3p,
            tc.tile_pool(name="c3t", bufs=3) as c3t,
        ):
            for h in range(2):
                bufA = c3b.tile([128, 32 * 198 + 4], BF16, tag="A")
                bufB = c3b.tile([64, 32 * 198 + 4], BF16, tag="B")
                for ck in range(2):
                    off = ck * 16 * 198
                    nc.sync.dma_start(
                        _v(bufA, 0, 128, [[1, 16 * 198]], off),
                        _v(xpad3, 64 * h, 64, [[1, 2], [1, 16 * 198]], off),
                    )
                    nc.sync.dma_start(
                        _v(bufB, 0, 64, [[1, 16 * 198]], off),
                        _v(xpad3, 64 * h, 64, [[1, 16 * 198]], off + 2),
                    )
                for bq in range(4):
                    stg = c3t.tile([128, 288], BF16, tag="stg")
                    for bj in range(4):
                        bp = bq * 4 + bj
                        ps = c3p.tile([128, 288], F32, tag="ps3")
                        for dy in range(3):
                            dims = [[198, 2], [14, 12], [1, 12]]
                            off = bp * 2 * 198 + dy * 14
                            nc.tensor.matmul(
                                ps[:, :], w3asb[0:128, dy * 128 : dy * 128 + 128],
                                _v(bufA, 0, 128, dims, off),
                                start=(dy == 0), stop=False,
                            )
                            nc.tensor.matmul(
                                ps[:, :], w3bsb[0:64, dy * 128 : dy * 128 + 128],
                                _v(bufB, 0, 64, dims, off),
                                start=False, stop=(dy == 2),
                            )
                        # pool (2b, 12y, 12x) -> (2b, 6y, 6x): one XY-reduce per sample
                        for i in range(2):
                            nc.vector.tensor_reduce(
                                _v(stg, 0, 128, [[6, 6], [1, 6]], bj * 72 + i * 36),
                                _v(ps, 0, 128, [[24, 6], [2, 6], [12, 2], [1, 2]], i * 144),
                                mybir.AxisListType.XY,
                                ALU.max,
                            )
                    dst = _v(feat, 0, 128, [[1, 288]], (h * 32 + bq * 8) * 36)
                    nc.scalar.activation(dst, _v(stg, 0, 128, [[1, 288]]), AF.Relu, bias=b3sb[:, 0:1])

        # ---------------- FC1 + LIF + FC2/FC3 ----------------
        with (
            tc.tile_pool(name="cur1p", bufs=1, space="PSUM") as cur1p,
            tc.tile_pool(name="cur2p", bufs=2, space="PSUM") as cur2p,
            tc.tile_pool(name="lif", bufs=1) as lifp,
            tc.tile_pool(name="liftmp", bufs=2) as dtp,
        ):
            psA = cur1p.tile([64, 512], F32)
            for k in range(36):
                nc.tensor.matmul(
                    psA[0:64, :],
                    _v(feat, 0, 128, [[36, 64]], k),
                    wf1sb[:, k * 512 : (k + 1) * 512],
                    start=(k == 0),
                    stop=(k == 35),
                )
            cur1sb = lifp.tile([64, 512], BF16)
            nc.scalar.activation(cur1sb[:, :], psA[0:64, :], AF.Copy)
            cur1 = cur1p.tile([128, 256], BF16)
            for g in range(4):
                nc.tensor.matmul(
                    cur1[:, 64 * g : 64 * g + 64],
                    cur1sb[0:64, 128 * g : 128 * g + 128],
                    identsb[0:64, 0:64],
                    is_transpose=True,
                    start=True,
                    stop=True,
                )

            zeros256 = lifp.tile([128, 256], F32)
            nc.gpsimd.memset(zeros256[:, :], 0.0)
            v1 = lifp.tile([128, 256], F32)
            s1 = lifp.tile([128, 256], BF16)
            nc.gpsimd.memset(v1[:, :], 0.0)
            v2 = lifp.tile([128, 64], F32)
            nc.gpsimd.memset(v2[:, :], 0.0)
            s2 = lifp.tile([128, 64], BF16)
            v3 = lifp.tile([5, 64], F32)
            nc.gpsimd.memset(v3[:, :], 0.0)
            acc = lifp.tile([5, 64], F32)
            nc.gpsimd.memset(acc[:, :], 0.0)

            def lif_step(v, cur, s_out):
                # v <- v + (cur - v)*0.5 ; s = (v >= 1) ; v <- 0 where s
                n = v.shape[1]
                d = dtp.tile([v.shape[0], n], F32, tag="d", name="d")
                nc.vector.tensor_tensor(d[:, :], cur[:, :], v[:, :], ALU.subtract)
                nc.vector.scalar_tensor_tensor(v[:, :], d[:, :], 0.5, v[:, :], ALU.mult, ALU.add)
                nc.vector.tensor_scalar(s_out[:, :], v[:, :], 1.0, None, ALU.is_ge)
                mask = s_out[:, :].bitcast(mybir.dt.uint16 if s_out.dtype == BF16 else mybir.dt.uint32)
                nc.vector.copy_predicated(v[:, :], mask, zeros256[0 : v.shape[0], 0 : n])

            for t in range(3):
                lif_step(v1, cur1, s1)
                cur2 = cur2p.tile([128, 64], F32, tag="cur2")
                for g in range(4):
                    nc.tensor.matmul(
                        cur2[:, :], wf2sb[:, g * 128 : g * 128 + 128], s1[:, 64 * g : 64 * g + 64],
                        start=(g == 0), stop=(g == 3),
                    )
                lif_step(v2, cur2, s2)
                cur3 = cur2p.tile([5, 64], F32, tag="cur3")
                nc.tensor.matmul(cur3[0:5, :], wf3sb[0:128, 0:5], s2[:, :], start=True, stop=True)
                s3 = dtp.tile([5, 64], F32, tag="s3")
                d3 = dtp.tile([5, 64], F32, tag="d3")
                nc.vector.tensor_tensor(d3[:, :], cur3[0:5, :], v3[:, :], ALU.subtract)
                nc.vector.scalar_tensor_tensor(v3[:, :], d3[:, :], 0.5, v3[:, :], ALU.mult, ALU.add)
                nc.vector.tensor_scalar(s3[:, :], v3[:, :], 1.0, None, ALU.is_ge)
                nc.vector.copy_predicated(v3[:, :], s3[:, :].bitcast(mybir.dt.uint32), zeros256[0:5, 0:64])
                nc.vector.tensor_tensor(acc[:, :], acc[:, :], s3[:, :], ALU.add)

            # acc/3 for acc in {0,1,2,3}: mult by fp32(1/3) matches true division except acc=3
            # (3*0.33333334 = 1.0000001) -> clamp with min(., 1.0) for exactness.
            nc.vector.tensor_scalar(acc[:, :], acc[:, :], float(np.float32(1.0) / np.float32(3.0)), 1.0, ALU.mult, ALU.min)
            nc.sync.dma_start(_dv(io["out"], 0, [[64, 5], [1, 64]]), acc[:, :])


def _build():
    nc = bacc.Bacc("TRN2", target_bir_lowering=False, debug=False, enable_asserts=True)
    io = {}

    def inp(name, shape, dt):
        io[name] = nc.dram_tensor(name, shape, dt, kind="ExternalInput").ap()

    inp("xpad", [3, 160128], BF16)
    inp("w1l", [128, 32], BF16)
    inp("w2l", [96, 192], BF16)
    inp("w3a", [128, 384], BF16)
    inp("w3b", [64, 384], BF16)
    inp("wf1", [128, 18432], BF16)
    inp("wf2", [128, 512], BF16)
    inp("wf3", [128, 5], BF16)
    inp("ident", [64, 64], BF16)
    inp("b1", [128, 1], F32)
    inp("b2", [128, 1], F32)
    inp("b3", [128, 1], F32)
    io["out"] = nc.dram_tensor("out", [5, 64], F32, kind="ExternalOutput").ap()

    import os
    unroll = int(os.environ.get("KERNEL_UNROLL", "1"))
    with tile.TileContext(nc) as tc:
        for _ in range(unroll):
            _emit(tc, io)
    nc.compile()
    return nc


def _fake_quant(w):
    w = np.asarray(w, np.float32)
    scale = np.float32(np.max(np.abs(w)) / np.float32(127.0))
    wq = np.clip(np.round(w / scale), -127.0, 127.0).astype(np.float32) * scale
    return wq.astype(np.float32)


def _bf16(a):
    return np.asarray(a, np.float32).astype(ml_dtypes.bfloat16)


def _prep_weights(conv1_w, conv1_b, conv2_w, conv2_b, conv3_w, conv3_b, W1, W2, W3):
    c1 = np.asarray(conv1_w, np.float32)  # [32, 3, 3, 3]
    c2 = np.asarray(conv2_w, np.float32)  # [64, 32, 3, 3]
    c3 = np.asarray(conv3_w, np.float32)  # [128, 64, 3, 3]

    w1l = np.zeros((128, 32), np.float32)
    wk = c1.transpose(2, 3, 1, 0).reshape(27, 32)  # [(dy,dx,c), m]
    for q in range(4):
        w1l[32 * q : 32 * q + 27] = wk

    w2l = c2.transpose(1, 3, 2, 0).reshape(96, 192)  # [(c,dx), (dy,m)]
    w3x = c3.transpose(1, 3, 2, 0)  # [c, dx, dy, m]
    w3a = w3x[:, 0:2].reshape(128, 384)
    w3b = w3x[:, 2].reshape(64, 384)

    W1q = _fake_quant(W1)  # [512, 4608]
    W2q = _fake_quant(W2)  # [128, 512]
    W3q = _fake_quant(W3)  # [5, 128]

    # [c, k*512 + u] = W1q[u, c*36 + k]  (FC1 computes [sample, unit], transposed after)
    wf1 = W1q.reshape(512, 128, 36).transpose(1, 2, 0).reshape(128, 36 * 512)
    wf2 = W2q.T.reshape(4, 128, 128).transpose(1, 0, 2).reshape(128, 512)
    wf3 = W3q.T.copy()  # [128, 5]

    return {
        "w1l": _bf16(w1l),
        "w2l": _bf16(w2l),
        "w3a": _bf16(w3a),
        "w3b": _bf16(w3b),
        "wf1": _bf16(wf1),
        "wf2": _bf16(wf2),
        "wf3": _bf16(wf3),
        "ident": _bf16(np.eye(64, dtype=np.float32)),
        "b1": np.tile(np.asarray(conv1_b, np.float32), 4).reshape(128, 1).copy(),
        "b2": np.tile(np.asarray(conv2_b, np.float32), 2).reshape(128, 1).copy(),
        "b3": np.asarray(conv3_b, np.float32).reshape(128, 1).copy(),
    }


_NC = None
LAST_RESULTS = None


def kernel(x, conv1_w, conv1_b, conv2_w, conv2_b, conv3_w, conv3_b, W1, W2, W3, _trace=False):
    global _NC, LAST_RESULTS
    if _NC is None:
        _NC = _build()

    wmap = _prep_weights(conv1_w, conv1_b, conv2_w, conv2_b, conv3_w, conv3_b, W1, W2, W3)

    x = np.asarray(x, np.float32)
    xp = np.zeros((512, 3, 50, 50), np.float32)
    xp[:, :, 1:49, 1:49] = x
    in_maps = []
    for i in range(NCORES):
        shard = xp[B * i : B * (i + 1)].transpose(1, 0, 2, 3).reshape(3, B * 2500)
        sp = np.zeros((3, 160128), np.float32)
        sp[:, : B * 2500] = shard
        in_maps.append({"xpad": _bf16(sp), **wmap})

    from concourse.bass_utils import run_bass_kernel_spmd

    res = run_bass_kernel_spmd(_NC, in_maps, core_ids=list(range(NCORES)), trace=_trace)
    LAST_RESULTS = res
    out = np.concatenate([np.asarray(res.results[i]["out"]).T for i in range(NCORES)], axis=0)
    return np.ascontiguousarray(out.astype(np.float32))


# revision 20
# speedup vs baseline: 402.1400x; 1.0301x over previous
"""Trainium2 Bass kernel for ConvFCNet (3x conv+pool -> int8-fakequant FC + LIF SNN head).

Data-parallel over 8 NeuronCores: batch 512 -> 64 samples/core, weights replicated.

Per-core pipeline (all activations bf16, PSUM accumulation fp32, LIF state fp32):
  conv1 3->32  48x48, pad1 + relu + maxpool2 -> [32, 24, 24]
      im2col (27 = 3c x 9 taps) built by DMA into 4 partition groups of 32,
      4 concurrent diagonal tile_position matmuls (K=27, M=32).
  conv2 32->64 24x24 -> [64, 12, 12]
      im2col over dx only (96 = 32c x 3dx); dy folded into matmul free-dim offsets;
      3 accumulating K=96 matmuls, 2 sample-halves run in parallel via col tiling.
  conv3 64->128 12x12 -> [128, 6, 6]
      im2col over dx: A=[128 = 64c x dx01], B=[64 = c, dx2]; 6 accumulating matmuls.
  FC1 4608->512 per-unit-group (4x128) stationary weights, feat chunks (hw-major) as rhs.
  LIF dynamics (tau=2, vth=1, hard reset) + FC2 512->128 + FC3 128->5, 3 timesteps,
      all in [unit, sample] orientation -> no transposes.
"""

import numpy as np
import ml_dtypes

import concourse.bass as bass
import concourse.bacc as bacc
import concourse.tile as tile
import concourse.mybir as mybir

AF = mybir.ActivationFunctionType
ALU = mybir.AluOpType
BF16 = mybir.dt.bfloat16
F32 = mybir.dt.float32

NCORES = 8
B = 64  # samples per core


def _v(ap, p0, npart, dims, off=0):
    """View into an SBUF/PSUM tile AP: partition slice [p0, p0+npart) + custom free dims."""
    pitch = ap.ap[0][0]
    return bass.AP(
        tensor=ap.tensor,
        offset=ap.offset + p0 * pitch + off,
        ap=[[pitch, npart]] + [list(d) for d in dims],
    )


def _dv(ap, off, dims):
    """View into a DRAM tensor AP with custom dims."""
    return bass.AP(tensor=ap.tensor, offset=ap.offset + off, ap=[list(d) for d in dims])


def _emit(tc, io):
    nc = tc.nc
    from contextlib import ExitStack

    with ExitStack() as ctx:
        # ---------------- persistent buffers + weights ----------------
        wp = ctx.enter_context(tc.tile_pool(name="wts", bufs=1))
        w1sb = wp.tile([128, 32], BF16)
        nc.gpsimd.dma_start(w1sb[:, :], io["w1l"][:, :])
        w2sb = wp.tile([96, 192], BF16)
        nc.gpsimd.dma_start(w2sb[:, :], io["w2l"][:, :])
        w3asb = wp.tile([128, 384], BF16)
        nc.gpsimd.dma_start(w3asb[:, :], io["w3a"][:, :])
        w3bsb = wp.tile([64, 384], BF16)
        nc.gpsimd.dma_start(w3bsb[:, :], io["w3b"][:, :])
        wf2sb = wp.tile([128, 512], BF16)
        nc.gpsimd.dma_start(wf2sb[:, :], io["wf2"][:, :])
        wf3sb = wp.tile([128, 5], BF16)
        nc.gpsimd.dma_start(wf3sb[:, :], io["wf3"][:, :])
        identsb = wp.tile([64, 64], BF16)
        nc.gpsimd.dma_start(identsb[:, :], io["ident"][:, :])
        b1sb = wp.tile([128, 1], F32)
        nc.gpsimd.dma_start(b1sb[:, :], io["b1"][:, :])
        b2sb = wp.tile([128, 1], F32)
        nc.gpsimd.dma_start(b2sb[:, :], io["b2"][:, :])
        b3sb = wp.tile([128, 1], F32)
        nc.gpsimd.dma_start(b3sb[:, :], io["b3"][:, :])

        mp = ctx.enter_context(tc.tile_pool(name="main", bufs=1))
        # conv1 pooled output, padded 26x26, partition 32q+c holds samples 16q..16q+15
        xpad2 = mp.tile([128, 16 * 676 + 4], BF16)
        for dims, off in [
            ([[676, 16], [1, 26]], 0),        # top row
            ([[676, 16], [1, 26]], 650),      # bottom row
            ([[676, 16], [26, 26]], 0),       # left col
            ([[676, 16], [26, 26]], 25),      # right col
            ([[1, 4]], 16 * 676),             # tail pad (im2col dx over-read)
        ]:
            nc.gpsimd.memset(_v(xpad2, 0, 128, dims, off), 0.0)
        # conv2 pooled output, padded 14x14, partition 64h+c holds samples 32h..32h+31
        xpad3 = mp.tile([128, 32 * 198 + 4], BF16)
        for dims, off in [
            ([[198, 32], [1, 14]], 0),        # top row
            ([[198, 32], [1, 14]], 182),      # bottom row
            ([[198, 32], [14, 14]], 0),       # left col
            ([[198, 32], [14, 14]], 13),      # right col
            ([[1, 4]], 32 * 198),             # tail pad (im2col dx over-read)
            ([[198, 32], [1, 2]], 196),       # per-sample slack (pitch 198 vs 196)
        ]:
            nc.gpsimd.memset(_v(xpad3, 0, 128, dims, off), 0.0)
        # conv3 pooled output (features): [128c, b*36 + hw]
        feat = mp.tile([128, B * 36], BF16)

        # ---------------- conv1 ----------------
        with (
            tc.tile_pool(name="c1imc", bufs=4) as c1i,
            tc.tile_pool(name="c1ps", bufs=4, space="PSUM") as c1p,
            tc.tile_pool(name="c1t", bufs=3) as c1t,
        ):
            for half in range(4):
                imc = c1i.tile([128, 4 * 2500], BF16, tag="imc")
                for q in range(4):
                    for dy in range(3):
                        src = _dv(
                            io["xpad"],
                            (q * 16 + half * 4) * 2500 + dy * 50,
                            [[1, 3], [160128, 3], [1, 10000]],
                        )
                        dst = _v(imc, 32 * q + dy * 9, 9, [[1, 10000]])
                        nc.sync.dma_start(dst, src)
                for s in range(4):
                    stg = c1t.tile([128, 576], BF16, tag="stg")
                    for yt in range(6):
                        ps = c1p.tile([128, 384], F32, tag="ps1")
                        for q in range(4):
                            rhs = _v(imc, 32 * q, 27, [[50, 8], [1, 48]], s * 2500 + yt * 400)
                            nc.tensor.matmul(
                                ps[32 * q : 32 * q + 32, :],
                                w1sb[32 * q : 32 * q + 27, 0:32],
                                rhs,
                                start=True,
                                stop=True,
                                tile_position=(32 * q, 32 * q),
                            )
                        # maxpool 2x2 on (8y, 48x) -> (4y, 24x) into the staging tile
                        nc.vector.tensor_reduce(
                            _v(stg, 0, 128, [[24, 4], [1, 24]], yt * 96),
                            _v(ps, 0, 128, [[96, 4], [2, 24], [48, 2], [1, 2]]),
                            mybir.AxisListType.XY,
                            ALU.max,
                        )
                    hs = half * 4 + s
                    dst = _v(xpad2, 0, 128, [[26, 24], [1, 24]], hs * 676 + 27)
                    nc.scalar.activation(dst, _v(stg, 0, 128, [[24, 24], [1, 24]]), AF.Relu, bias=b1sb[:, 0:1])

        # FC1 weights: start the 4.7MB load early so it overlaps conv2/conv3 compute
        fcw = ctx.enter_context(tc.tile_pool(name="fcw", bufs=1))
        wf1sb = fcw.tile([128, 18432], BF16)
        nc.gpsimd.dma_start(wf1sb[:, :], io["wf1"][:, :])

        # ---------------- conv2 ----------------
        with (
            tc.tile_pool(name="c2buf", bufs=1) as c2b,
            tc.tile_pool(name="c2ps", bufs=4, space="PSUM") as c2p,
            tc.tile_pool(name="c2t", bufs=3) as c2t,
        ):
            buf96 = c2b.tile([96, B * 676 + 4], BF16)
            for qt in range(4):
                for j in range(4):
                    src = _v(xpad2, 32 * j, 32, [[1, 3], [1, 4 * 676]], qt * 4 * 676)
                    dst = _v(buf96, 0, 96, [[1, 4 * 676]], (j * 16 + qt * 4) * 676)
                    nc.sync.dma_start(dst, src)
            for blk in range(8):
                stg = c2t.tile([128, 576], BF16, tag="stg")
                for bi in range(4):
                    b = blk * 4 + bi
                    for yh in range(2):
                        ps = c2p.tile([128, 288], F32, tag="ps2")
                        for h in range(2):
                            for dy in range(3):
                                rhs = _v(
                                    buf96, 0, 96, [[26, 12], [1, 24]],
                                    (h * 32 + b) * 676 + yh * 312 + dy * 26,
                                )
                                nc.tensor.matmul(
                                    ps[64 * h : 64 * h + 64, :],
                                    w2sb[0:96, dy * 64 : dy * 64 + 64],
                                    rhs,
                                    start=(dy == 0),
                                    stop=(dy == 2),
                                    tile_position=(0, 64 * h),
                                )
                        # pool (12y, 24x) -> (6y, 12x) into staging
                        nc.vector.tensor_reduce(
                            _v(stg, 0, 128, [[12, 6], [1, 12]], bi * 144 + yh * 72),
                            _v(ps, 0, 128, [[48, 6], [2, 12], [24, 2], [1, 2]]),
                            mybir.AxisListType.XY,
                            ALU.max,
                        )
                dst = _v(xpad3, 0, 128, [[198, 4], [14, 12], [1, 12]], blk * 4 * 198 + 15)
                nc.scalar.activation(dst, _v(stg, 0, 128, [[144, 4], [12, 12], [1, 12]]), AF.Relu, bias=b2sb[:, 0:1])

        # ---------------- conv3 ----------------
        with (
            tc.tile_pool(name="c3buf", bufs=2) as c3b,
            tc.tile_pool(name="c3ps", bufs=4, space="PSUM") as c3p,
            tc.tile_pool(name="c3t", bufs=3) as c3t,
        ):
            for h in range(2):
                bufA = c3b.tile([128, 32 * 198 + 4], BF16, tag="A")
                bufB = c3b.tile([64, 32 * 198 + 4], BF16, tag="B")
                for ck in range(2):
                    off = ck * 16 * 198
                    nc.sync.dma_start(
                        _v(bufA, 0, 128, [[1, 16 * 198]], off),
                        _v(xpad3, 64 * h, 64, [[1, 2], [1, 16 * 198]], off),
                    )
                    nc.sync.dma_start(
                        _v(bufB, 0, 64, [[1, 16 * 198]], off),
                        _v(xpad3, 64 * h, 64, [[1, 16 * 198]], off + 2),
                    )
                for bq in range(4):
                    stg = c3t.tile([128, 288], BF16, tag="stg")
                    for bj in range(4):
                        bp = bq * 4 + bj
                        ps = c3p.tile([128, 288], F32, tag="ps3")
                        for dy in range(3):
                            dims = [[198, 2], [14, 12], [1, 12]]
                            off = bp * 2 * 198 + dy * 14
                            nc.tensor.matmul(
                                ps[:, :], w3asb[0:128, dy * 128 : dy * 128 + 128],
                                _v(bufA, 0, 128, dims, off),
                                start=(dy == 0), stop=False,
                            )
                            nc.tensor.matmul(
                                ps[:, :], w3bsb[0:64, dy * 128 : dy * 128 + 128],
                                _v(bufB, 0, 64, dims, off),
                                start=False, stop=(dy == 2),
                            )
                        # pool (2b, 12y, 12x) -> (2b, 6y, 6x): one XY-reduce per sample
                        for i in range(2):
                            nc.vector.tensor_reduce(
                                _v(stg, 0, 128, [[6, 6], [1, 6]], bj * 72 + i * 36),
                                _v(ps, 0, 128, [[24, 6], [2, 6], [12, 2], [1, 2]], i * 144),
                                mybir.AxisListType.XY,
                                ALU.max,
                            )
                    dst = _v(feat, 0, 128, [[1, 288]], (h * 32 + bq * 8) * 36)
                    nc.scalar.activation(dst, _v(stg, 0, 128, [[1, 288]]), AF.Relu, bias=b3sb[:, 0:1])

        # ---------------- FC1 + LIF + FC2/FC3 ----------------
        with (
            tc.tile_pool(name="cur1p", bufs=1, space="PSUM") as cur1p,
            tc.tile_pool(name="cur2p", bufs=2, space="PSUM") as cur2p,
            tc.tile_pool(name="lif", bufs=1) as lifp,
            tc.tile_pool(name="liftmp", bufs=2) as dtp,
        ):
            psA = cur1p.tile([64, 512], F32)
            for k in range(36):
                nc.tensor.matmul(
                    psA[0:64, :],
                    _v(feat, 0, 128, [[36, 64]], k),
                    wf1sb[:, k * 512 : (k + 1) * 512],
                    start=(k == 0),
                    stop=(k == 35),
                )
            cur1sb = lifp.tile([64, 512], BF16)
            nc.scalar.activation(cur1sb[:, :], psA[0:64, :], AF.Copy)
            cur1 = cur1p.tile([128, 256], BF16)
            for g in range(4):
                nc.tensor.matmul(
                    cur1[:, 64 * g : 64 * g + 64],
                    cur1sb[0:64, 128 * g : 128 * g + 128],
                    identsb[0:64, 0:64],
                    is_transpose=True,
                    start=True,
                    stop=True,
                )

            zeros256 = lifp.tile([128, 256], F32)
            nc.gpsimd.memset(zeros256[:, :], 0.0)
            v1 = lifp.tile([128, 256], F32)
            s1 = lifp.tile([128, 256], BF16)
            nc.gpsimd.memset(v1[:, :], 0.0)
            v2 = lifp.tile([128, 64], F32)
            nc.gpsimd.memset(v2[:, :], 0.0)
            s2 = lifp.tile([128, 64], BF16)
            v3 = lifp.tile([5, 64], F32)
            nc.gpsimd.memset(v3[:, :], 0.0)
            acc = lifp.tile([5, 64], F32)
            nc.gpsimd.memset(acc[:, :], 0.0)

            def lif_step(v, cur, s_out):
                # v <- v + (cur - v)*0.5 ; s = (v >= 1) ; v <- 0 where s
                n = v.shape[1]
                d = dtp.tile([v.shape[0], n], F32, tag="d", name="d")
                nc.vector.tensor_tensor(d[:, :], cur[:, :], v[:, :], ALU.subtract)
                nc.vector.scalar_tensor_tensor(v[:, :], d[:, :], 0.5, v[:, :], ALU.mult, ALU.add)
                nc.vector.tensor_scalar(s_out[:, :], v[:, :], 1.0, None, ALU.is_ge)
                mask = s_out[:, :].bitcast(mybir.dt.uint16 if s_out.dtype == BF16 else mybir.dt.uint32)
                nc.vector.copy_predicated(v[:, :], mask, zeros256[0 : v.shape[0], 0 : n])

            for t in range(3):
                lif_step(v1, cur1, s1)
                cur2 = cur2p.tile([128, 64], F32, tag="cur2")
                for g in range(4):
                    nc.tensor.matmul(
                        cur2[:, :], wf2sb[:, g * 128 : g * 128 + 128], s1[:, 64 * g : 64 * g + 64],
                        start=(g == 0), stop=(g == 3),
                    )
                lif_step(v2, cur2, s2)
                cur3 = cur2p.tile([5, 64], F32, tag="cur3")
                nc.tensor.matmul(cur3[0:5, :], wf3sb[0:128, 0:5], s2[:, :], start=True, stop=True)
                s3 = dtp.tile([5, 64], F32, tag="s3")
                d3 = dtp.tile([5, 64], F32, tag="d3")
                nc.vector.tensor_tensor(d3[:, :], cur3[0:5, :], v3[:, :], ALU.subtract)
                nc.vector.scalar_tensor_tensor(v3[:, :], d3[:, :], 0.5, v3[:, :], ALU.mult, ALU.add)
                nc.vector.tensor_scalar(s3[:, :], v3[:, :], 1.0, None, ALU.is_ge)
                nc.vector.copy_predicated(v3[:, :], s3[:, :].bitcast(mybir.dt.uint32), zeros256[0:5, 0:64])
                nc.vector.tensor_tensor(acc[:, :], acc[:, :], s3[:, :], ALU.add)

            # acc/3 for acc in {0,1,2,3}: mult by fp32(1/3) matches true division except acc=3
            # (3*0.33333334 = 1.0000001) -> clamp with min(., 1.0) for exactness.
            nc.vector.tensor_scalar(acc[:, :], acc[:, :], float(np.float32(1.0) / np.float32(3.0)), 1.0, ALU.mult, ALU.min)
            nc.sync.dma_start(_dv(io["out"], 0, [[64, 5], [1, 64]]), acc[:, :])


def _build():
    nc = bacc.Bacc("TRN2", target_bir_lowering=False, debug=False, enable_asserts=True)
    io = {}

    def inp(name, shape, dt):
        io[name] = nc.dram_tensor(name, shape, dt, kind="ExternalInput").ap()

    inp("xpad", [3, 160128], BF16)
    inp("w1l", [128, 32], BF16)
    inp("w2l", [96, 192], BF16)
    inp("w3a", [128, 384], BF16)
    inp("w3b", [64, 384], BF16)
    inp("wf1", [128, 18432], BF16)
    inp("wf2", [128, 512], BF16)
    inp("wf3", [128, 5], BF16)
    inp("ident", [64, 64], BF16)
    inp("b1", [128, 1], F32)
    inp("b2", [128, 1], F32)
    inp("b3", [128, 1], F32)
    io["out"] = nc.dram_tensor("out", [5, 64], F32, kind="ExternalOutput").ap()

    import os
    unroll = int(os.environ.get("KERNEL_UNROLL", "1"))
    with tile.TileContext(nc) as tc:
        for _ in range(unroll):
            _emit(tc, io)
    nc.compile()
    return nc


def _fake_quant(w):
    w = np.asarray(w, np.float32)
    scale = np.float32(np.max(np.abs(w)) / np.float32(127.0))
    wq = np.clip(np.round(w / scale), -127.0, 127.0).astype(np.float32) * scale
    return wq.astype(np.float32)


def _bf16(a):
    return np.asarray(a, np.float32).astype(ml_dtypes.bfloat16)


def _prep_weights(conv1_w, conv1_b, conv2_w, conv2_b, conv3_w, conv3_b, W1, W2, W3):
    c1 = np.asarray(conv1_w, np.float32)  # [32, 3, 3, 3]
    c2 = np.asarray(conv2_w, np.float32)  # [64, 32, 3, 3]
    c3 = np.asarray(conv3_w, np.float32)  # [128, 64, 3, 3]

    w1l = np.zeros((128, 32), np.float32)
    wk = c1.transpose(2, 3, 1, 0).reshape(27, 32)  # [(dy,dx,c), m]
    for q in range(4):
        w1l[32 * q : 32 * q + 27] = wk

    w2l = c2.transpose(1, 3, 2, 0).reshape(96, 192)  # [(c,dx), (dy,m)]
    w3x = c3.transpose(1, 3, 2, 0)  # [c, dx, dy, m]
    w3a = w3x[:, 0:2].reshape(128, 384)
    w3b = w3x[:, 2].reshape(64, 384)

    W1q = _fake_quant(W1)  # [512, 4608]
    W2q = _fake_quant(W2)  # [128, 512]
    W3q = _fake_quant(W3)  # [5, 128]

    # [c, k*512 + u] = W1q[u, c*36 + k]  (FC1 computes [sample, unit], transposed after)
    wf1 = W1q.reshape(512, 128, 36).transpose(1, 2, 0).reshape(128, 36 * 512)
    wf2 = W2q.T.reshape(4, 128, 128).transpose(1, 0, 2).reshape(128, 512)
    wf3 = W3q.T.copy()  # [128, 5]

    return {
        "w1l": _bf16(w1l),
        "w2l": _bf16(w2l),
        "w3a": _bf16(w3a),
        "w3b": _bf16(w3b),
        "wf1": _bf16(wf1),
        "wf2": _bf16(wf2),
        "wf3": _bf16(wf3),
        "ident": _bf16(np.eye(64, dtype=np.float32)),
        "b1": np.tile(np.asarray(conv1_b, np.float32), 4).reshape(128, 1).copy(),
        "b2": np.tile(np.asarray(conv2_b, np.float32), 2).reshape(128, 1).copy(),
        "b3": np.asarray(conv3_b, np.float32).reshape(128, 1).copy(),
    }


_NC = None
LAST_RESULTS = None


def kernel(x, conv1_w, conv1_b, conv2_w, conv2_b, conv3_w, conv3_b, W1, W2, W3, _trace=False):
    global _NC, LAST_RESULTS
    if _NC is None:
        _NC = _build()

    wmap = _prep_weights(conv1_w, conv1_b, conv2_w, conv2_b, conv3_w, conv3_b, W1, W2, W3)

    x = np.asarray(x, np.float32)
    xp = np.zeros((512, 3, 50, 50), np.float32)
    xp[:, :, 1:49, 1:49] = x
    in_maps = []
    for i in range(NCORES):
        shard = xp[B * i : B * (i + 1)].transpose(1, 0, 2, 3).reshape(3, B * 2500)
        sp = np.zeros((3, 160128), np.float32)
        sp[:, : B * 2500] = shard
        in_maps.append({"xpad": _bf16(sp), **wmap})

    from concourse.bass_utils import run_bass_kernel_spmd

    res = run_bass_kernel_spmd(_NC, in_maps, core_ids=list(range(NCORES)), trace=_trace)
    LAST_RESULTS = res
    out = np.concatenate([np.asarray(res.results[i]["out"]).T for i in range(NCORES)], axis=0)
    return np.ascontiguousarray(out.astype(np.float32))
